# revision 12
# baseline (speedup 1.0000x reference)
"""Trainium2 Bass kernel for nn_AttentionLayer (sparse_attention).

Math (per batch b, history l):
    info = [q, k, q-k, q*k] @ W1 + b1 ; @ W2 + b2 ; sigmoid ; @ Wf + bf
    score = softmax(where(mask, -inf, logit), axis=l)
    out   = sum_l score * v

Host-side algebra (exact up to fp assoc):
  - No nonlinearity between W1/W2  =>  h2 = k@P + (q*k)@Q + r_b
        P = (W1b-W1c)@W2, Q = W1d@W2, r_b = q_b@(W1a+W1c)@W2 + b1@W2 + b2
  - Fold q into per-batch weights: h2 = k @ V_b + r_b,  V_b = P + diag(q_b) Q
  - Fold r_b into k: solve s_b @ V_b = r_b (least-norm), ship k + s_b
  - sigmoid(x)@Wf = tanh(x/2)@(Wf/2) + const; const cancels in softmax
  - MASK COMPACTION: masked tokens (exp(-inf)=0) are dropped on host; each
    batch's <=126 unmasked tokens are packed into 128 slots (pads: k=0 ->
    logit 0, madd=-30, v=0). Halves k/v traffic and all device compute.
Device layout: token-major 2-stream columns (batch-pair r -> 128 cols,
partitions 0:64 = stream-A E-dims, 64:128 = stream-B). One block-diagonal
[128,80] matmul per pair -> h2 [80,128] (A h2 parts 0:40, B 40:80); tanh
(scale .5) -> t bf16; wf matmuls [80,2] write logits into 4 PSUM partition
strips (32s, 32s+1) at N=512; ACT-copy evacuates [98,512] to bf16 staging;
8 strided DMAs per quarter land logits batch-major [128,128]; softmax + p@v
on DVE (exp w/ accum z on ACT; mult + 2 folds + reduce + scale).
"""

import sys

sys.path.insert(0, "/opt/trn_rl_repo")

import numpy as np
import ml_dtypes

import concourse.bass as bass
import concourse.bacc as bacc
import concourse.tile as tile
import concourse.mybir as mybir
from concourse.bass_utils import run_bass_kernel_spmd

N_CORES = 8
B_FULL = 4096
B = B_FULL // N_CORES  # 512 batches per core
E = 64
H = 40
LP = 128               # compacted history slots per batch
NPAIR = B // 2         # 256 batch pairs per core
NGRP = 16              # pairs per group (one h2 psum tile)
NSLAB = 8              # kx/vw DMA slabs (32 pairs each)

BF16 = mybir.dt.bfloat16
F32 = mybir.dt.float32
nbf16 = ml_dtypes.bfloat16


def build_nc():
    nc = bacc.Bacc()

    kx_d = nc.declare_dram_parameter("kx", [128, NPAIR * LP], BF16, isOutput=False)
    vw_d = nc.declare_dram_parameter("vw", [128, NPAIR * 80], BF16, isOutput=False)
    wf_d = nc.declare_dram_parameter("wf2", [80, 2], BF16, isOutput=False)
    vt_d = nc.declare_dram_parameter("vt", [B, E * LP], BF16, isOutput=False)
    madd_d = nc.declare_dram_parameter("madd", [B, LP], BF16, isOutput=False)
    out_d = nc.declare_dram_parameter("out", [B, E], F32, isOutput=True)

    Tanh = mybir.ActivationFunctionType.Tanh
    Exp = mybir.ActivationFunctionType.Exp
    Copy = mybir.ActivationFunctionType.Copy
    Alu = mybir.AluOpType
    X = mybir.AxisListType.X

    SLABC = 2 * NGRP * LP   # kx cols per slab (4096)
    SLABW = 2 * NGRP * 80   # vw cols per slab (2560)

    from contextlib import ExitStack

    with tile.TileContext(nc) as tc, ExitStack() as ctx:
        const = ctx.enter_context(tc.tile_pool(name="const", bufs=1))
        kxp = ctx.enter_context(tc.tile_pool(name="kxp", bufs=2))
        vwp = ctx.enter_context(tc.tile_pool(name="vwp", bufs=2))
        h2p = ctx.enter_context(tc.tile_pool(name="h2p", bufs=2, space="PSUM"))
        lgp = ctx.enter_context(tc.tile_pool(name="lgp", bufs=2, space="PSUM"))
        tp = ctx.enter_context(tc.tile_pool(name="tp", bufs=2))
        stp = ctx.enter_context(tc.tile_pool(name="stp", bufs=2))
        lmp = ctx.enter_context(tc.tile_pool(name="lmp", bufs=2))
        vtp = ctx.enter_context(tc.tile_pool(name="vtp", bufs=2))
        mp = ctx.enter_context(tc.tile_pool(name="mp", bufs=2))
        bp = ctx.enter_context(tc.tile_pool(name="bp", bufs=2))

        wf_t = const.tile([80, 2], BF16, tag="wf")
        nc.sync.dma_start(wf_t[:], wf_d[:])

        kx_t = {}
        vw_t = {}

        # variable slabs (in groups of 8 pairs): small first slabs so the
        # first matmul starts after ~256KB of DMA instead of 1.6MB
        SLAB_GROUPS = [1, 1, 2, 4, 4, 4, 4, 4, 4, 4]
        SLAB_G0 = np.cumsum([0] + SLAB_GROUPS).tolist()

        def load_slab(s):
            ng = SLAB_GROUPS[s]
            g0 = SLAB_G0[s]
            kt = kxp.tile([128, 4 * 8 * LP], BF16, tag="kx", name=f"kx{s}")
            nc.sync.dma_start(kt[:, 0:ng * 8 * LP],
                              kx_d[:, g0 * 8 * LP:(g0 + ng) * 8 * LP])
            kx_t[s] = kt
            wt = vwp.tile([128, 4 * 8 * 80], BF16, tag="vw", name=f"vw{s}")
            nc.sync.dma_start(wt[:, 0:ng * 8 * 80],
                              vw_d[:, g0 * 8 * 80:(g0 + ng) * 8 * 80])
            vw_t[s] = wt

        qdat = {}

        def load_quarter(qq):
            # on the same (sync) queue as the kx/vw slabs: FIFO order on one
            # ring makes input arrival match consumption order exactly
            vt_t = vtp.tile([128, E * LP], BF16, tag="vt", name=f"vt{qq}")
            nc.sync.dma_start(vt_t[:], vt_d[qq * 128:(qq + 1) * 128, :])
            md_t = mp.tile([128, LP], BF16, tag="md", name=f"md{qq}")
            nc.sync.dma_start(md_t[:], madd_d[qq * 128:(qq + 1) * 128, :])
            qdat[qq] = (vt_t, md_t)

        def phase_b(qq, lm_t):
            vt_t, md_t = qdat.pop(qq)
            ladj = bp.tile([128, LP], F32, tag="ladj", name=f"ladj{qq}")
            nc.vector.tensor_tensor(ladj[:], lm_t[:], md_t[:], Alu.add)
            p_t = bp.tile([128, LP], BF16, tag="p", name=f"p{qq}")
            z_t = bp.tile([128, 1], F32, tag="z", name=f"z{qq}")
            nc.scalar.activation(p_t[:], ladj[:], Exp, accum_out=z_t[:])

            w1 = bp.tile([128, E * LP], BF16, tag="w1", name=f"w1{qq}")
            p_b = p_t[:].rearrange("p (o l) -> p o l", o=1).broadcast_to([128, E, LP])
            nc.vector.tensor_tensor(
                w1[:].rearrange("p (e l) -> p e l", e=E),
                vt_t[:].rearrange("p (e l) -> p e l", e=E),
                p_b, Alu.mult,
            )
            w2 = bp.tile([128, E * LP // 2], BF16, tag="w2", name=f"w2{qq}")
            w1v = w1[:].rearrange("p (e l) -> p e l", e=E)
            nc.vector.tensor_tensor(
                w2[:].rearrange("p (e l) -> p e l", e=E),
                w1v[:, :, 0:LP // 2], w1v[:, :, LP // 2:LP], Alu.add,
            )
            w3 = bp.tile([128, E * LP // 4], BF16, tag="w3", name=f"w3{qq}")
            w2v = w2[:].rearrange("p (e l) -> p e l", e=E)
            nc.vector.tensor_tensor(
                w3[:].rearrange("p (e l) -> p e l", e=E),
                w2v[:, :, 0:LP // 4], w2v[:, :, LP // 4:LP // 2], Alu.add,
            )
            acc = bp.tile([128, E], F32, tag="acc", name=f"acc{qq}")
            nc.vector.tensor_reduce(
                acc[:], w3[:].rearrange("p (e l) -> p e l", e=E), axis=X, op=Alu.add)
            rz = bp.tile([128, 1], F32, tag="rz", name=f"rz{qq}")
            nc.vector.reciprocal(rz[:], z_t[:])
            o_t = bp.tile([128, E], F32, tag="o", name=f"o{qq}")
            nc.vector.tensor_scalar_mul(o_t[:], acc[:], rz[:])
            nc.gpsimd.dma_start(out_d[qq * 128:(qq + 1) * 128, :], o_t[:])

        load_slab(0)
        st_t = None
        lg_t = None
        GP = 8  # pairs per h2 group ([80, 1024] f32 = 2 psum banks)
        slab_of_group = []
        for si, ng in enumerate(SLAB_GROUPS):
            slab_of_group += [si] * ng
        for g in range(32):
            s = slab_of_group[g]
            if g == SLAB_G0[s] and s + 1 < len(SLAB_GROUPS):
                load_slab(s + 1)
            if g == 2:
                load_quarter(0)
            if g % 8 == 0 and g >= 8:
                load_quarter(g // 8)

            kxs, vws = kx_t[s], vw_t[s]
            h2_t = h2p.tile([80, GP * LP], F32, tag="h2", name=f"h2_{g}")
            for pp in range(GP):
                rr = (g - SLAB_G0[s]) * GP + pp  # pair within slab
                nc.tensor.matmul(
                    h2_t[0:80, pp * LP:(pp + 1) * LP],
                    vws[:, rr * 80:rr * 80 + 80],
                    kxs[:, rr * LP:(rr + 1) * LP],
                    start=True, stop=True,
                )
            t_t = tp.tile([80, GP * LP], BF16, tag="t", name=f"t_{g}")
            nc.scalar.activation(t_t[:], h2_t[:], Tanh, scale=0.5)

            if g % 2 == 0:
                lg_t = lgp.tile([98, 512], F32, tag="lg", name=f"lg_{g // 2}")
            for j in range(2):
                ss = 2 * (g % 2) + j
                nc.tensor.matmul(
                    lg_t[32 * ss:32 * ss + 2, 0:512],
                    wf_t[:], t_t[:, 512 * j:512 * (j + 1)],
                    start=True, stop=True, tile_position=(0, 32 * ss),
                )
            if g % 2 == 1:
                qq, gq = g // 8, (g // 2) % 4
                if gq == 0:
                    st_t = stp.tile([98, 4 * 512], BF16, tag="st", name=f"st{qq}")
                nc.scalar.activation(
                    st_t[:, 512 * gq:512 * (gq + 1)], lg_t[:], Copy)

                if gq == 3:
                    lm_t = lmp.tile([128, LP], BF16, tag="lm", name=f"lm{qq}")
                    for ss in range(4):
                        for sig in range(2):
                            row = 32 * ss + sig
                            dr = 16 * (2 * ss + sig)
                            nc.gpsimd.dma_start(
                                lm_t[dr:dr + 16, :], st_t[row:row + 1, :])
                    phase_b(qq, lm_t)

    if not nc.is_finalized():
        nc.finalize()
    return nc


def host_prep(q, k, v, mask, W1, b1, W2, b2, Wf, bf):
    """Fold weights per batch, compact masked tokens, build device layouts."""
    q2 = q[:, 0, :].astype(np.float32)
    W1 = W1.astype(np.float32); W2 = W2.astype(np.float32)
    P = (W1[64:128] - W1[128:192]) @ W2                     # [64,40]
    Q = W1[192:256] @ W2                                    # [64,40]
    A2 = (W1[0:64] + W1[128:192]) @ W2
    c0 = b1.astype(np.float32) @ W2 + b2.astype(np.float32)
    r = q2 @ A2 + c0                                        # [Bf,40]
    V = P[None] + q2[:, :, None] * Q[None]                  # [Bf,64,40]
    G = np.einsum('beh,bei->bhi', V, V)
    y = np.linalg.solve(G, r[:, :, None])
    s = np.einsum('beh,bhx->be', V, y)                      # [Bf,64]

    m = mask[:, :, 0]
    order = np.argsort(m, axis=1, kind='stable')[:, :LP]
    nvalid = (~m).sum(1)
    assert nvalid.max() <= LP, f"batch with {nvalid.max()} unmasked tokens"
    validc = np.arange(LP)[None, :] < nvalid[:, None]       # [Bf,LP]
    kc = np.take_along_axis(k.astype(np.float32), order[:, :, None], 1)
    vc = np.take_along_axis(v.astype(np.float32), order[:, :, None], 1)
    kc = np.where(validc[..., None], kc + s[:, None, :], 0.0)
    vc = np.where(validc[..., None], vc, 0.0)
    maddf = np.where(validc, np.float32(0.0), np.float32(-30.0)).astype(nbf16)

    # core-local batch <-> (pair r, stream sig) map
    b = np.arange(B)
    qq = b // 128; t = b % 128
    s2s = t // 16; s_ = s2s // 2; sig = s2s % 2
    g_ = (t % 16) // 4; cb = t % 4
    r_ = 64 * qq + 16 * g_ + 4 * s_ + cb
    A_idx = np.empty(NPAIR, np.int64); B_idx = np.empty(NPAIR, np.int64)
    A_idx[r_[sig == 0]] = b[sig == 0]
    B_idx[r_[sig == 1]] = b[sig == 1]

    in_maps = []
    for c in range(N_CORES):
        sl = slice(c * B, (c + 1) * B)
        kcc, Vc = kc[sl], V[sl]
        kx = np.empty((128, NPAIR * LP), np.float32)
        kx[0:64] = kcc[A_idx].transpose(2, 0, 1).reshape(64, -1)
        kx[64:128] = kcc[B_idx].transpose(2, 0, 1).reshape(64, -1)
        vw3 = np.zeros((NPAIR, 128, 80), np.float32)
        vw3[:, 0:64, 0:40] = Vc[A_idx]
        vw3[:, 64:128, 40:80] = Vc[B_idx]
        vw = vw3.transpose(1, 0, 2).reshape(128, NPAIR * 80)
        vt = np.ascontiguousarray(vc[sl].transpose(0, 2, 1)).reshape(B, E * LP)
        wf2 = np.zeros((80, 2), np.float32)
        wf2[0:40, 0] = 0.5 * Wf[:, 0]
        wf2[40:80, 1] = 0.5 * Wf[:, 0]
        in_maps.append({
            "kx": np.ascontiguousarray(kx).astype(nbf16),
            "vw": np.ascontiguousarray(vw).astype(nbf16),
            "wf2": wf2.astype(nbf16),
            "vt": vt.astype(nbf16),
            "madd": np.ascontiguousarray(maddf[sl]),
        })
    return in_maps


_CACHE = {}


def run_on_device(in_maps, trace=False):
    if "nc" not in _CACHE:
        _CACHE["nc"] = build_nc()
    nc = _CACHE["nc"]
    res = run_bass_kernel_spmd(nc, in_maps, core_ids=list(range(N_CORES)),
                               trace=trace)
    return res


def kernel(q, k, v, mask, W1, b1, W2, b2, Wf, bf):
    in_maps = host_prep(q, k, v, mask, W1, b1, W2, b2, Wf, bf)
    res = run_on_device(in_maps)
    out = np.concatenate([res.results[c]["out"] for c in range(N_CORES)], axis=0)
    return out.astype(np.float32)


# revision 15
# speedup vs baseline: 1.0354x; 1.0354x over previous
"""Trainium2 Bass kernel for nn_AttentionLayer (sparse_attention).

Math (per batch b, history l):
    info = [q, k, q-k, q*k] @ W1 + b1 ; @ W2 + b2 ; sigmoid ; @ Wf + bf
    score = softmax(where(mask, -inf, logit), axis=l)
    out   = sum_l score * v

Host-side algebra (exact up to fp assoc):
  - No nonlinearity between W1/W2  =>  h2 = k@P + (q*k)@Q + r_b
        P = (W1b-W1c)@W2, Q = W1d@W2, r_b = q_b@(W1a+W1c)@W2 + b1@W2 + b2
  - Fold q into per-batch weights: h2 = k @ V_b + r_b,  V_b = P + diag(q_b) Q
  - Fold r_b into k: solve s_b @ V_b = r_b (least-norm), ship k + s_b
  - sigmoid(x)@Wf = tanh(x/2)@(Wf/2) + const; const cancels in softmax
  - MASK COMPACTION: masked tokens (exp(-inf)=0) are dropped on host; each
    batch's <=126 unmasked tokens are packed into 128 slots (pads: k=0 ->
    logit 0, madd=-30, v=0). Halves k/v traffic and all device compute.
Device layout: token-major 2-stream columns (batch-pair r -> 128 cols,
partitions 0:64 = stream-A E-dims, 64:128 = stream-B). One block-diagonal
[128,80] matmul per pair -> h2 [80,128] (A h2 parts 0:40, B 40:80); tanh
(scale .5) -> t bf16; wf matmuls [80,2] write logits into 4 PSUM partition
strips (32s, 32s+1) at N=512; ACT-copy evacuates [98,512] to bf16 staging;
8 strided DMAs per quarter land logits batch-major [128,128]; softmax + p@v
on DVE (exp w/ accum z on ACT; mult + 2 folds + reduce + scale).
"""

import sys

sys.path.insert(0, "/opt/trn_rl_repo")

import numpy as np
import ml_dtypes

import concourse.bass as bass
import concourse.bacc as bacc
import concourse.tile as tile
import concourse.mybir as mybir
from concourse.bass_utils import run_bass_kernel_spmd

N_CORES = 8
B_FULL = 4096
B = B_FULL // N_CORES  # 512 batches per core
E = 64
H = 40
LP = 128               # compacted history slots per batch
NPAIR = B // 2         # 256 batch pairs per core
NGRP = 16              # pairs per group (one h2 psum tile)
NSLAB = 8              # kx/vw DMA slabs (32 pairs each)

BF16 = mybir.dt.bfloat16
F32 = mybir.dt.float32
nbf16 = ml_dtypes.bfloat16


def build_nc():
    nc = bacc.Bacc()

    kx_d = nc.declare_dram_parameter("kx", [128, NPAIR * LP], BF16, isOutput=False)
    vw_d = nc.declare_dram_parameter("vw", [128, NPAIR * 80], BF16, isOutput=False)
    wf_d = nc.declare_dram_parameter("wf2", [80, 2], BF16, isOutput=False)
    vt_d = nc.declare_dram_parameter("vt", [B, E * LP], BF16, isOutput=False)
    madd_d = nc.declare_dram_parameter("madd", [B, LP], BF16, isOutput=False)
    out_d = nc.declare_dram_parameter("out", [B, E], F32, isOutput=True)

    Tanh = mybir.ActivationFunctionType.Tanh
    Exp = mybir.ActivationFunctionType.Exp
    Copy = mybir.ActivationFunctionType.Copy
    Alu = mybir.AluOpType
    X = mybir.AxisListType.X

    SLABC = 2 * NGRP * LP   # kx cols per slab (4096)
    SLABW = 2 * NGRP * 80   # vw cols per slab (2560)

    from contextlib import ExitStack

    with tile.TileContext(nc) as tc, ExitStack() as ctx:
        const = ctx.enter_context(tc.tile_pool(name="const", bufs=1))
        kxp = ctx.enter_context(tc.tile_pool(name="kxp", bufs=2))
        vwp = ctx.enter_context(tc.tile_pool(name="vwp", bufs=2))
        h2p = ctx.enter_context(tc.tile_pool(name="h2p", bufs=2, space="PSUM"))
        lgp = ctx.enter_context(tc.tile_pool(name="lgp", bufs=2, space="PSUM"))
        tp = ctx.enter_context(tc.tile_pool(name="tp", bufs=2))
        stp = ctx.enter_context(tc.tile_pool(name="stp", bufs=2))
        lmp = ctx.enter_context(tc.tile_pool(name="lmp", bufs=2))
        vtp = ctx.enter_context(tc.tile_pool(name="vtp", bufs=2))
        mp = ctx.enter_context(tc.tile_pool(name="mp", bufs=2))
        bp = ctx.enter_context(tc.tile_pool(name="bp", bufs=2))

        wf_t = const.tile([80, 2], BF16, tag="wf")
        nc.sync.dma_start(wf_t[:], wf_d[:])

        kx_t = {}
        vw_t = {}

        # variable slabs (in groups of 8 pairs): small first slabs so the
        # first matmul starts after ~256KB of DMA instead of 1.6MB
        SLAB_GROUPS = [1, 1, 2, 4, 4, 4, 4, 4, 4, 4]
        SLAB_G0 = np.cumsum([0] + SLAB_GROUPS).tolist()

        def load_slab(s):
            ng = SLAB_GROUPS[s]
            g0 = SLAB_G0[s]
            kt = kxp.tile([128, 4 * 8 * LP], BF16, tag="kx", name=f"kx{s}")
            nc.sync.dma_start(kt[:, 0:ng * 8 * LP],
                              kx_d[:, g0 * 8 * LP:(g0 + ng) * 8 * LP])
            kx_t[s] = kt
            wt = vwp.tile([128, 4 * 8 * 80], BF16, tag="vw", name=f"vw{s}")
            nc.sync.dma_start(wt[:, 0:ng * 8 * 80],
                              vw_d[:, g0 * 8 * 80:(g0 + ng) * 8 * 80])
            vw_t[s] = wt

        qdat = {}

        def load_quarter(qq, chunk):
            # same (sync) ring as the kx/vw slabs so arrival order matches
            # consumption order, but in 512KB chunks interleaved between
            # slab loads so the slab stream never stalls behind a 2MB blob
            if chunk == 0:
                vt_t = vtp.tile([128, E * LP], BF16, tag="vt", name=f"vt{qq}")
                md_t = mp.tile([128, LP], BF16, tag="md", name=f"md{qq}")
                nc.sync.dma_start(md_t[:], madd_d[qq * 128:(qq + 1) * 128, :])
                qdat[qq] = (vt_t, md_t)
            vt_t = qdat[qq][0]
            c0 = chunk * (E * LP // 4)
            c1 = (chunk + 1) * (E * LP // 4)
            nc.sync.dma_start(vt_t[:, c0:c1], vt_d[qq * 128:(qq + 1) * 128, c0:c1])

        def phase_b(qq, lm_t):
            vt_t, md_t = qdat.pop(qq)
            ladj = bp.tile([128, LP], F32, tag="ladj", name=f"ladj{qq}")
            nc.vector.tensor_tensor(ladj[:], lm_t[:], md_t[:], Alu.add)
            p_t = bp.tile([128, LP], BF16, tag="p", name=f"p{qq}")
            z_t = bp.tile([128, 1], F32, tag="z", name=f"z{qq}")
            nc.scalar.activation(p_t[:], ladj[:], Exp, accum_out=z_t[:])

            w1 = bp.tile([128, E * LP], BF16, tag="w1", name=f"w1{qq}")
            p_b = p_t[:].rearrange("p (o l) -> p o l", o=1).broadcast_to([128, E, LP])
            nc.vector.tensor_tensor(
                w1[:].rearrange("p (e l) -> p e l", e=E),
                vt_t[:].rearrange("p (e l) -> p e l", e=E),
                p_b, Alu.mult,
            )
            w2 = bp.tile([128, E * LP // 2], BF16, tag="w2", name=f"w2{qq}")
            w1v = w1[:].rearrange("p (e l) -> p e l", e=E)
            nc.vector.tensor_tensor(
                w2[:].rearrange("p (e l) -> p e l", e=E),
                w1v[:, :, 0:LP // 2], w1v[:, :, LP // 2:LP], Alu.add,
            )
            w3 = bp.tile([128, E * LP // 4], BF16, tag="w3", name=f"w3{qq}")
            w2v = w2[:].rearrange("p (e l) -> p e l", e=E)
            nc.vector.tensor_tensor(
                w3[:].rearrange("p (e l) -> p e l", e=E),
                w2v[:, :, 0:LP // 4], w2v[:, :, LP // 4:LP // 2], Alu.add,
            )
            acc = bp.tile([128, E], F32, tag="acc", name=f"acc{qq}")
            nc.vector.tensor_reduce(
                acc[:], w3[:].rearrange("p (e l) -> p e l", e=E), axis=X, op=Alu.add)
            rz = bp.tile([128, 1], F32, tag="rz", name=f"rz{qq}")
            nc.vector.reciprocal(rz[:], z_t[:])
            o_t = bp.tile([128, E], F32, tag="o", name=f"o{qq}")
            nc.vector.tensor_scalar_mul(o_t[:], acc[:], rz[:])
            nc.gpsimd.dma_start(out_d[qq * 128:(qq + 1) * 128, :], o_t[:])

        load_slab(0)
        st_t = None
        lg_t = None
        GP = 8  # pairs per h2 group ([80, 1024] f32 = 2 psum banks)
        slab_of_group = []
        for si, ng in enumerate(SLAB_GROUPS):
            slab_of_group += [si] * ng
        for g in range(32):
            s = slab_of_group[g]
            if g == SLAB_G0[s] and s + 1 < len(SLAB_GROUPS):
                load_slab(s + 1)
            if 2 <= g < 6:
                load_quarter(0, g - 2)
            elif g >= 8 and g % 8 < 4:
                load_quarter(g // 8, g % 8)

            kxs, vws = kx_t[s], vw_t[s]
            h2_t = h2p.tile([80, GP * LP], F32, tag="h2", name=f"h2_{g}")
            for pp in range(GP):
                rr = (g - SLAB_G0[s]) * GP + pp  # pair within slab
                nc.tensor.matmul(
                    h2_t[0:80, pp * LP:(pp + 1) * LP],
                    vws[:, rr * 80:rr * 80 + 80],
                    kxs[:, rr * LP:(rr + 1) * LP],
                    start=True, stop=True,
                )
            t_t = tp.tile([80, GP * LP], BF16, tag="t", name=f"t_{g}")
            nc.scalar.activation(t_t[:], h2_t[:], Tanh, scale=0.5)

            if g % 2 == 0:
                lg_t = lgp.tile([98, 512], F32, tag="lg", name=f"lg_{g // 2}")
            for j in range(2):
                ss = 2 * (g % 2) + j
                nc.tensor.matmul(
                    lg_t[32 * ss:32 * ss + 2, 0:512],
                    wf_t[:], t_t[:, 512 * j:512 * (j + 1)],
                    start=True, stop=True, tile_position=(0, 32 * ss),
                )
            if g % 2 == 1:
                qq, gq = g // 8, (g // 2) % 4
                if gq == 0:
                    st_t = stp.tile([98, 4 * 512], BF16, tag="st", name=f"st{qq}")
                nc.scalar.activation(
                    st_t[:, 512 * gq:512 * (gq + 1)], lg_t[:], Copy)

                if gq == 3:
                    lm_t = lmp.tile([128, LP], BF16, tag="lm", name=f"lm{qq}")
                    for ss in range(4):
                        # rows {32s, 32s+1} unfold to batch-major rows
                        # 32s..32s+32 in one DMA (row-major both sides)
                        nc.gpsimd.dma_start(
                            lm_t[32 * ss:32 * ss + 32, :],
                            st_t[32 * ss:32 * ss + 2, :])
                    phase_b(qq, lm_t)

    if not nc.is_finalized():
        nc.finalize()
    return nc


def host_prep(q, k, v, mask, W1, b1, W2, b2, Wf, bf):
    """Fold weights per batch, compact masked tokens, build device layouts."""
    q2 = q[:, 0, :].astype(np.float32)
    W1 = W1.astype(np.float32); W2 = W2.astype(np.float32)
    P = (W1[64:128] - W1[128:192]) @ W2                     # [64,40]
    Q = W1[192:256] @ W2                                    # [64,40]
    A2 = (W1[0:64] + W1[128:192]) @ W2
    c0 = b1.astype(np.float32) @ W2 + b2.astype(np.float32)
    r = q2 @ A2 + c0                                        # [Bf,40]
    V = P[None] + q2[:, :, None] * Q[None]                  # [Bf,64,40]
    G = np.einsum('beh,bei->bhi', V, V)
    y = np.linalg.solve(G, r[:, :, None])
    s = np.einsum('beh,bhx->be', V, y)                      # [Bf,64]

    m = mask[:, :, 0]
    order = np.argsort(m, axis=1, kind='stable')[:, :LP]
    nvalid = (~m).sum(1)
    assert nvalid.max() <= LP, f"batch with {nvalid.max()} unmasked tokens"
    validc = np.arange(LP)[None, :] < nvalid[:, None]       # [Bf,LP]
    kc = np.take_along_axis(k.astype(np.float32), order[:, :, None], 1)
    vc = np.take_along_axis(v.astype(np.float32), order[:, :, None], 1)
    kc = np.where(validc[..., None], kc + s[:, None, :], 0.0)
    vc = np.where(validc[..., None], vc, 0.0)
    maddf = np.where(validc, np.float32(0.0), np.float32(-30.0)).astype(nbf16)

    # core-local batch <-> (pair r, stream sig) map
    b = np.arange(B)
    qq = b // 128; t = b % 128
    s2s = t // 16; s_ = s2s // 2; sig = s2s % 2
    g_ = (t % 16) // 4; cb = t % 4
    r_ = 64 * qq + 16 * g_ + 4 * s_ + cb
    A_idx = np.empty(NPAIR, np.int64); B_idx = np.empty(NPAIR, np.int64)
    A_idx[r_[sig == 0]] = b[sig == 0]
    B_idx[r_[sig == 1]] = b[sig == 1]

    in_maps = []
    for c in range(N_CORES):
        sl = slice(c * B, (c + 1) * B)
        kcc, Vc = kc[sl], V[sl]
        kx = np.empty((128, NPAIR * LP), np.float32)
        kx[0:64] = kcc[A_idx].transpose(2, 0, 1).reshape(64, -1)
        kx[64:128] = kcc[B_idx].transpose(2, 0, 1).reshape(64, -1)
        vw3 = np.zeros((NPAIR, 128, 80), np.float32)
        vw3[:, 0:64, 0:40] = Vc[A_idx]
        vw3[:, 64:128, 40:80] = Vc[B_idx]
        vw = vw3.transpose(1, 0, 2).reshape(128, NPAIR * 80)
        vt = np.ascontiguousarray(vc[sl].transpose(0, 2, 1)).reshape(B, E * LP)
        wf2 = np.zeros((80, 2), np.float32)
        wf2[0:40, 0] = 0.5 * Wf[:, 0]
        wf2[40:80, 1] = 0.5 * Wf[:, 0]
        in_maps.append({
            "kx": np.ascontiguousarray(kx).astype(nbf16),
            "vw": np.ascontiguousarray(vw).astype(nbf16),
            "wf2": wf2.astype(nbf16),
            "vt": vt.astype(nbf16),
            "madd": np.ascontiguousarray(maddf[sl]),
        })
    return in_maps


_CACHE = {}


def run_on_device(in_maps, trace=False):
    if "nc" not in _CACHE:
        _CACHE["nc"] = build_nc()
    nc = _CACHE["nc"]
    res = run_bass_kernel_spmd(nc, in_maps, core_ids=list(range(N_CORES)),
                               trace=trace)
    return res


def kernel(q, k, v, mask, W1, b1, W2, b2, Wf, bf):
    in_maps = host_prep(q, k, v, mask, W1, b1, W2, b2, Wf, bf)
    res = run_on_device(in_maps)
    out = np.concatenate([res.results[c]["out"] for c in range(N_CORES)], axis=0)
    return out.astype(np.float32)


# revision 16
# speedup vs baseline: 1.0491x; 1.0132x over previous
"""Trainium2 Bass kernel for nn_AttentionLayer (sparse_attention).

Math (per batch b, history l):
    info = [q, k, q-k, q*k] @ W1 + b1 ; @ W2 + b2 ; sigmoid ; @ Wf + bf
    score = softmax(where(mask, -inf, logit), axis=l)
    out   = sum_l score * v

Host-side algebra (exact up to fp assoc):
  - No nonlinearity between W1/W2  =>  h2 = k@P + (q*k)@Q + r_b
        P = (W1b-W1c)@W2, Q = W1d@W2, r_b = q_b@(W1a+W1c)@W2 + b1@W2 + b2
  - Fold q into per-batch weights: h2 = k @ V_b + r_b,  V_b = P + diag(q_b) Q
  - Fold r_b into k: solve s_b @ V_b = r_b (least-norm), ship k + s_b
  - sigmoid(x)@Wf = tanh(x/2)@(Wf/2) + const; const cancels in softmax
  - MASK COMPACTION: masked tokens (exp(-inf)=0) are dropped on host; each
    batch's <=126 unmasked tokens are packed into 128 slots (pads: k=0 ->
    logit 0, madd=-30, v=0). Halves k/v traffic and all device compute.
Device layout: token-major 2-stream columns (batch-pair r -> 128 cols,
partitions 0:64 = stream-A E-dims, 64:128 = stream-B). One block-diagonal
[128,80] matmul per pair -> h2 [80,128] (A h2 parts 0:40, B 40:80); tanh
(scale .5) -> t bf16; wf matmuls [80,2] write logits into 4 PSUM partition
strips (32s, 32s+1) at N=512; ACT-copy evacuates [98,512] to bf16 staging;
8 strided DMAs per quarter land logits batch-major [128,128]; softmax + p@v
on DVE (exp w/ accum z on ACT; mult + 2 folds + reduce + scale).
"""

import sys

sys.path.insert(0, "/opt/trn_rl_repo")

import numpy as np
import ml_dtypes

import concourse.bass as bass
import concourse.bacc as bacc
import concourse.tile as tile
import concourse.mybir as mybir
from concourse.bass_utils import run_bass_kernel_spmd

N_CORES = 8
B_FULL = 4096
B = B_FULL // N_CORES  # 512 batches per core
E = 64
H = 40
LP = 128               # compacted history slots per batch
NPAIR = B // 2         # 256 batch pairs per core
NGRP = 16              # pairs per group (one h2 psum tile)
NSLAB = 8              # kx/vw DMA slabs (32 pairs each)

BF16 = mybir.dt.bfloat16
F32 = mybir.dt.float32
nbf16 = ml_dtypes.bfloat16


def build_nc():
    nc = bacc.Bacc()

    kx_d = nc.declare_dram_parameter("kx", [128, NPAIR * LP], BF16, isOutput=False)
    vw_d = nc.declare_dram_parameter("vw", [128, NPAIR * 80], BF16, isOutput=False)
    wf_d = nc.declare_dram_parameter("wf2", [80, 2], BF16, isOutput=False)
    vt_d = nc.declare_dram_parameter("vt", [B, E * LP], BF16, isOutput=False)
    madd_d = nc.declare_dram_parameter("madd", [B, LP], BF16, isOutput=False)
    out_d = nc.declare_dram_parameter("out", [B, E], F32, isOutput=True)

    Tanh = mybir.ActivationFunctionType.Tanh
    Exp = mybir.ActivationFunctionType.Exp
    Copy = mybir.ActivationFunctionType.Copy
    Alu = mybir.AluOpType
    X = mybir.AxisListType.X

    SLABC = 2 * NGRP * LP   # kx cols per slab (4096)
    SLABW = 2 * NGRP * 80   # vw cols per slab (2560)

    from contextlib import ExitStack

    with tile.TileContext(nc) as tc, ExitStack() as ctx:
        const = ctx.enter_context(tc.tile_pool(name="const", bufs=1))
        kxp = ctx.enter_context(tc.tile_pool(name="kxp", bufs=2))
        vwp = ctx.enter_context(tc.tile_pool(name="vwp", bufs=2))
        h2p = ctx.enter_context(tc.tile_pool(name="h2p", bufs=2, space="PSUM"))
        lgp = ctx.enter_context(tc.tile_pool(name="lgp", bufs=2, space="PSUM"))
        tp = ctx.enter_context(tc.tile_pool(name="tp", bufs=2))
        stp = ctx.enter_context(tc.tile_pool(name="stp", bufs=2))
        lmp = ctx.enter_context(tc.tile_pool(name="lmp", bufs=2))
        vtp = ctx.enter_context(tc.tile_pool(name="vtp", bufs=3))
        mp = ctx.enter_context(tc.tile_pool(name="mp", bufs=2))
        bp = ctx.enter_context(tc.tile_pool(name="bp", bufs=2))

        wf_t = const.tile([80, 2], BF16, tag="wf")
        nc.sync.dma_start(wf_t[:], wf_d[:])

        kx_t = {}
        vw_t = {}

        # variable slabs (in groups of 8 pairs): small first slabs so the
        # first matmul starts after ~256KB of DMA instead of 1.6MB
        SLAB_GROUPS = [1, 1, 2, 4, 4, 4, 4, 4, 4, 4]
        SLAB_G0 = np.cumsum([0] + SLAB_GROUPS).tolist()

        def load_slab(s):
            ng = SLAB_GROUPS[s]
            g0 = SLAB_G0[s]
            kt = kxp.tile([128, 4 * 8 * LP], BF16, tag="kx", name=f"kx{s}")
            nc.sync.dma_start(kt[:, 0:ng * 8 * LP],
                              kx_d[:, g0 * 8 * LP:(g0 + ng) * 8 * LP])
            kx_t[s] = kt
            wt = vwp.tile([128, 4 * 8 * 80], BF16, tag="vw", name=f"vw{s}")
            nc.sync.dma_start(wt[:, 0:ng * 8 * 80],
                              vw_d[:, g0 * 8 * 80:(g0 + ng) * 8 * 80])
            vw_t[s] = wt

        qdat = {}

        def load_quarter(qq, chunk):
            # same (sync) ring as the kx/vw slabs so arrival order matches
            # consumption order, but in 512KB chunks interleaved between
            # slab loads so the slab stream never stalls behind a 2MB blob
            if chunk == 0:
                vt_t = vtp.tile([128, E * LP], BF16, tag="vt", name=f"vt{qq}")
                md_t = mp.tile([128, LP], BF16, tag="md", name=f"md{qq}")
                nc.sync.dma_start(md_t[:], madd_d[qq * 128:(qq + 1) * 128, :])
                qdat[qq] = (vt_t, md_t)
            vt_t = qdat[qq][0]
            c0 = chunk * (E * LP // 4)
            c1 = (chunk + 1) * (E * LP // 4)
            nc.sync.dma_start(vt_t[:, c0:c1], vt_d[qq * 128:(qq + 1) * 128, c0:c1])

        def phase_b(qq, lm_t):
            vt_t, md_t = qdat.pop(qq)
            ladj = bp.tile([128, LP], F32, tag="ladj", name=f"ladj{qq}")
            nc.vector.tensor_tensor(ladj[:], lm_t[:], md_t[:], Alu.add)
            p_t = bp.tile([128, LP], BF16, tag="p", name=f"p{qq}")
            z_t = bp.tile([128, 1], F32, tag="z", name=f"z{qq}")
            nc.scalar.activation(p_t[:], ladj[:], Exp, accum_out=z_t[:])

            w1 = bp.tile([128, E * LP], BF16, tag="w1", name=f"w1{qq}")
            p_b = p_t[:].rearrange("p (o l) -> p o l", o=1).broadcast_to([128, E, LP])
            nc.vector.tensor_tensor(
                w1[:].rearrange("p (e l) -> p e l", e=E),
                vt_t[:].rearrange("p (e l) -> p e l", e=E),
                p_b, Alu.mult,
            )
            w2 = bp.tile([128, E * LP // 2], BF16, tag="w2", name=f"w2{qq}")
            w1v = w1[:].rearrange("p (e l) -> p e l", e=E)
            nc.vector.tensor_tensor(
                w2[:].rearrange("p (e l) -> p e l", e=E),
                w1v[:, :, 0:LP // 2], w1v[:, :, LP // 2:LP], Alu.add,
            )
            w3 = bp.tile([128, E * LP // 4], BF16, tag="w3", name=f"w3{qq}")
            w2v = w2[:].rearrange("p (e l) -> p e l", e=E)
            nc.vector.tensor_tensor(
                w3[:].rearrange("p (e l) -> p e l", e=E),
                w2v[:, :, 0:LP // 4], w2v[:, :, LP // 4:LP // 2], Alu.add,
            )
            acc = bp.tile([128, E], F32, tag="acc", name=f"acc{qq}")
            nc.vector.tensor_reduce(
                acc[:], w3[:].rearrange("p (e l) -> p e l", e=E), axis=X, op=Alu.add)
            rz = bp.tile([128, 1], F32, tag="rz", name=f"rz{qq}")
            nc.vector.reciprocal(rz[:], z_t[:])
            o_t = bp.tile([128, E], F32, tag="o", name=f"o{qq}")
            nc.vector.tensor_scalar_mul(o_t[:], acc[:], rz[:])
            nc.gpsimd.dma_start(out_d[qq * 128:(qq + 1) * 128, :], o_t[:])

        load_slab(0)
        st_t = None
        lg_t = None
        GP = 8  # pairs per h2 group ([80, 1024] f32 = 2 psum banks)
        slab_of_group = []
        for si, ng in enumerate(SLAB_GROUPS):
            slab_of_group += [si] * ng
        for g in range(32):
            s = slab_of_group[g]
            if g == SLAB_G0[s] and s + 1 < len(SLAB_GROUPS):
                load_slab(s + 1)
            if 2 <= g < 6:
                load_quarter(0, g - 2)
            elif g >= 8 and g % 8 < 4:
                load_quarter(g // 8, g % 8)

            kxs, vws = kx_t[s], vw_t[s]
            h2_t = h2p.tile([80, GP * LP], F32, tag="h2", name=f"h2_{g}")
            for pp in range(GP):
                rr = (g - SLAB_G0[s]) * GP + pp  # pair within slab
                nc.tensor.matmul(
                    h2_t[0:80, pp * LP:(pp + 1) * LP],
                    vws[:, rr * 80:rr * 80 + 80],
                    kxs[:, rr * LP:(rr + 1) * LP],
                    start=True, stop=True,
                )
            t_t = tp.tile([80, GP * LP], BF16, tag="t", name=f"t_{g}")
            nc.scalar.activation(t_t[:], h2_t[:], Tanh, scale=0.5)

            if g % 2 == 0:
                lg_t = lgp.tile([98, 512], F32, tag="lg", name=f"lg_{g // 2}")
            for j in range(2):
                ss = 2 * (g % 2) + j
                nc.tensor.matmul(
                    lg_t[32 * ss:32 * ss + 2, 0:512],
                    wf_t[:], t_t[:, 512 * j:512 * (j + 1)],
                    start=True, stop=True, tile_position=(0, 32 * ss),
                )
            if g % 2 == 1:
                qq, gq = g // 8, (g // 2) % 4
                if gq == 0:
                    st_t = stp.tile([98, 4 * 512], BF16, tag="st", name=f"st{qq}")
                nc.scalar.activation(
                    st_t[:, 512 * gq:512 * (gq + 1)], lg_t[:], Copy)

                if gq == 3:
                    lm_t = lmp.tile([128, LP], BF16, tag="lm", name=f"lm{qq}")
                    for ss in range(4):
                        # rows {32s, 32s+1} unfold to batch-major rows
                        # 32s..32s+32 in one DMA (row-major both sides)
                        nc.gpsimd.dma_start(
                            lm_t[32 * ss:32 * ss + 32, :],
                            st_t[32 * ss:32 * ss + 2, :])
                    phase_b(qq, lm_t)

    if not nc.is_finalized():
        nc.finalize()
    return nc


def host_prep(q, k, v, mask, W1, b1, W2, b2, Wf, bf):
    """Fold weights per batch, compact masked tokens, build device layouts."""
    q2 = q[:, 0, :].astype(np.float32)
    W1 = W1.astype(np.float32); W2 = W2.astype(np.float32)
    P = (W1[64:128] - W1[128:192]) @ W2                     # [64,40]
    Q = W1[192:256] @ W2                                    # [64,40]
    A2 = (W1[0:64] + W1[128:192]) @ W2
    c0 = b1.astype(np.float32) @ W2 + b2.astype(np.float32)
    r = q2 @ A2 + c0                                        # [Bf,40]
    V = P[None] + q2[:, :, None] * Q[None]                  # [Bf,64,40]
    G = np.einsum('beh,bei->bhi', V, V)
    y = np.linalg.solve(G, r[:, :, None])
    s = np.einsum('beh,bhx->be', V, y)                      # [Bf,64]

    m = mask[:, :, 0]
    order = np.argsort(m, axis=1, kind='stable')[:, :LP]
    nvalid = (~m).sum(1)
    assert nvalid.max() <= LP, f"batch with {nvalid.max()} unmasked tokens"
    validc = np.arange(LP)[None, :] < nvalid[:, None]       # [Bf,LP]
    kc = np.take_along_axis(k.astype(np.float32), order[:, :, None], 1)
    vc = np.take_along_axis(v.astype(np.float32), order[:, :, None], 1)
    kc = np.where(validc[..., None], kc + s[:, None, :], 0.0)
    vc = np.where(validc[..., None], vc, 0.0)
    maddf = np.where(validc, np.float32(0.0), np.float32(-30.0)).astype(nbf16)

    # core-local batch <-> (pair r, stream sig) map
    b = np.arange(B)
    qq = b // 128; t = b % 128
    s2s = t // 16; s_ = s2s // 2; sig = s2s % 2
    g_ = (t % 16) // 4; cb = t % 4
    r_ = 64 * qq + 16 * g_ + 4 * s_ + cb
    A_idx = np.empty(NPAIR, np.int64); B_idx = np.empty(NPAIR, np.int64)
    A_idx[r_[sig == 0]] = b[sig == 0]
    B_idx[r_[sig == 1]] = b[sig == 1]

    in_maps = []
    for c in range(N_CORES):
        sl = slice(c * B, (c + 1) * B)
        kcc, Vc = kc[sl], V[sl]
        kx = np.empty((128, NPAIR * LP), np.float32)
        kx[0:64] = kcc[A_idx].transpose(2, 0, 1).reshape(64, -1)
        kx[64:128] = kcc[B_idx].transpose(2, 0, 1).reshape(64, -1)
        vw3 = np.zeros((NPAIR, 128, 80), np.float32)
        vw3[:, 0:64, 0:40] = Vc[A_idx]
        vw3[:, 64:128, 40:80] = Vc[B_idx]
        vw = vw3.transpose(1, 0, 2).reshape(128, NPAIR * 80)
        vt = np.ascontiguousarray(vc[sl].transpose(0, 2, 1)).reshape(B, E * LP)
        wf2 = np.zeros((80, 2), np.float32)
        wf2[0:40, 0] = 0.5 * Wf[:, 0]
        wf2[40:80, 1] = 0.5 * Wf[:, 0]
        in_maps.append({
            "kx": np.ascontiguousarray(kx).astype(nbf16),
            "vw": np.ascontiguousarray(vw).astype(nbf16),
            "wf2": wf2.astype(nbf16),
            "vt": vt.astype(nbf16),
            "madd": np.ascontiguousarray(maddf[sl]),
        })
    return in_maps


_CACHE = {}


def run_on_device(in_maps, trace=False):
    if "nc" not in _CACHE:
        _CACHE["nc"] = build_nc()
    nc = _CACHE["nc"]
    res = run_bass_kernel_spmd(nc, in_maps, core_ids=list(range(N_CORES)),
                               trace=trace)
    return res


def kernel(q, k, v, mask, W1, b1, W2, b2, Wf, bf):
    in_maps = host_prep(q, k, v, mask, W1, b1, W2, b2, Wf, bf)
    res = run_on_device(in_maps)
    out = np.concatenate([res.results[c]["out"] for c in range(N_CORES)], axis=0)
    return out.astype(np.float32)


# revision 17
# speedup vs baseline: 1.1695x; 1.1147x over previous
"""Trainium2 Bass kernel for nn_AttentionLayer (sparse_attention).

Math (per batch b, history l):
    info = [q, k, q-k, q*k] @ W1 + b1 ; @ W2 + b2 ; sigmoid ; @ Wf + bf
    score = softmax(where(mask, -inf, logit), axis=l)
    out   = sum_l score * v

Host-side algebra (exact up to fp assoc):
  - No nonlinearity between W1/W2  =>  h2 = k@P + (q*k)@Q + r_b
        P = (W1b-W1c)@W2, Q = W1d@W2, r_b = q_b@(W1a+W1c)@W2 + b1@W2 + b2
  - Fold q into per-batch weights: h2 = k @ V_b + r_b,  V_b = P + diag(q_b) Q
  - Fold r_b into k: solve s_b @ V_b = r_b (least-norm), ship k + s_b
  - sigmoid(x)@Wf = tanh(x/2)@(Wf/2) + const; const cancels in softmax
  - MASK COMPACTION: masked tokens (exp(-inf)=0) are dropped on host; each
    batch's <=126 unmasked tokens are packed into 128 slots (pads: k=0 ->
    logit 0, madd=-30, v=0). Halves k/v traffic and all device compute.
Device layout: token-major 2-stream columns (batch-pair r -> 128 cols,
partitions 0:64 = stream-A E-dims, 64:128 = stream-B). One block-diagonal
[128,80] matmul per pair -> h2 [80,128] (A h2 parts 0:40, B 40:80); tanh
(scale .5) -> t bf16; wf matmuls [80,2] write logits into 4 PSUM partition
strips (32s, 32s+1) at N=512; ACT-copy evacuates [98,512] to bf16 staging;
8 strided DMAs per quarter land logits batch-major [128,128]; softmax + p@v
on DVE (exp w/ accum z on ACT; mult + 2 folds + reduce + scale).
"""

import sys

sys.path.insert(0, "/opt/trn_rl_repo")

import numpy as np
import ml_dtypes

import concourse.bass as bass
import concourse.bacc as bacc
import concourse.tile as tile
import concourse.mybir as mybir
from concourse.bass_utils import run_bass_kernel_spmd

N_CORES = 8
B_FULL = 4096
B = B_FULL // N_CORES  # 512 batches per core
E = 64
H = 40
LP = 128               # compacted history slots per batch
NPAIR = B // 2         # 256 batch pairs per core
NGRP = 16              # pairs per group (one h2 psum tile)
NSLAB = 8              # kx/vw DMA slabs (32 pairs each)

BF16 = mybir.dt.bfloat16
F32 = mybir.dt.float32
nbf16 = ml_dtypes.bfloat16


def build_nc():
    nc = bacc.Bacc()

    kx_d = nc.declare_dram_parameter("kx", [128, NPAIR * LP], BF16, isOutput=False)
    vw_d = nc.declare_dram_parameter("vw", [128, NPAIR * 80], BF16, isOutput=False)
    wf_d = nc.declare_dram_parameter("wf2", [80, 2], BF16, isOutput=False)
    vt_d = nc.declare_dram_parameter("vt", [B, E * LP], BF16, isOutput=False)
    madd_d = nc.declare_dram_parameter("madd", [B, LP], BF16, isOutput=False)
    out_d = nc.declare_dram_parameter("out", [B, E], F32, isOutput=True)

    Tanh = mybir.ActivationFunctionType.Tanh
    Exp = mybir.ActivationFunctionType.Exp
    Copy = mybir.ActivationFunctionType.Copy
    Alu = mybir.AluOpType
    X = mybir.AxisListType.X

    SLABC = 2 * NGRP * LP   # kx cols per slab (4096)
    SLABW = 2 * NGRP * 80   # vw cols per slab (2560)

    from contextlib import ExitStack

    with tile.TileContext(nc) as tc, ExitStack() as ctx:
        const = ctx.enter_context(tc.tile_pool(name="const", bufs=1))
        kxp = ctx.enter_context(tc.tile_pool(name="kxp", bufs=1))
        vwp = ctx.enter_context(tc.tile_pool(name="vwp", bufs=1))
        h2p = ctx.enter_context(tc.tile_pool(name="h2p", bufs=2, space="PSUM"))
        lgp = ctx.enter_context(tc.tile_pool(name="lgp", bufs=2, space="PSUM"))
        tp = ctx.enter_context(tc.tile_pool(name="tp", bufs=2))
        stp = ctx.enter_context(tc.tile_pool(name="stp", bufs=1))
        lmp = ctx.enter_context(tc.tile_pool(name="lmp", bufs=2))
        vtp = ctx.enter_context(tc.tile_pool(name="vtp", bufs=2))
        mp = ctx.enter_context(tc.tile_pool(name="mp", bufs=2))
        bp = ctx.enter_context(tc.tile_pool(name="bp", bufs=1))

        wf_t = const.tile([80, 2], BF16, tag="wf")
        nc.sync.dma_start(wf_t[:], wf_d[:])

        kx_t = {}
        vw_t = {}

        # variable slabs (in groups of 8 pairs): small first slabs so the
        # first matmul starts after ~256KB of DMA instead of 1.6MB
        SLAB_GROUPS = [1, 1, 2, 4, 4, 4, 4, 4, 4, 4]
        SLAB_G0 = np.cumsum([0] + SLAB_GROUPS).tolist()

        def load_slab(s):
            ng = SLAB_GROUPS[s]
            g0 = SLAB_G0[s]
            kt = kxp.tile([128, ng * 8 * LP], BF16, tag=f"kx{s}", name=f"kx{s}")
            nc.sync.dma_start(kt[:], kx_d[:, g0 * 8 * LP:(g0 + ng) * 8 * LP])
            kx_t[s] = kt
            wt = vwp.tile([128, ng * 8 * 80], BF16, tag=f"vw{s}", name=f"vw{s}")
            nc.sync.dma_start(wt[:], vw_d[:, g0 * 8 * 80:(g0 + ng) * 8 * 80])
            vw_t[s] = wt

        qdat = {}

        def load_quarter(qq, chunk):
            # same (sync) ring as the kx/vw slabs so arrival order matches
            # consumption order, but in 512KB chunks interleaved between
            # slab loads so the slab stream never stalls behind a 2MB blob
            if chunk == 0:
                vt_t = vtp.tile([128, E * LP], BF16, tag="vt", name=f"vt{qq}")
                md_t = mp.tile([128, LP], BF16, tag="md", name=f"md{qq}")
                nc.gpsimd.dma_start(md_t[:], madd_d[qq * 128:(qq + 1) * 128, :])
                qdat[qq] = (vt_t, md_t)
            vt_t = qdat[qq][0]
            c0 = chunk * (E * LP // 4)
            c1 = (chunk + 1) * (E * LP // 4)
            nc.gpsimd.dma_start(vt_t[:, c0:c1], vt_d[qq * 128:(qq + 1) * 128, c0:c1])

        def phase_b(qq, lm_t):
            vt_t, md_t = qdat.pop(qq)
            ladj = bp.tile([128, LP], F32, tag="ladj", name=f"ladj{qq}")
            nc.vector.tensor_tensor(ladj[:], lm_t[:], md_t[:], Alu.add)
            p_t = bp.tile([128, LP], BF16, tag="p", name=f"p{qq}")
            z_t = bp.tile([128, 1], F32, tag="z", name=f"z{qq}")
            nc.scalar.activation(p_t[:], ladj[:], Exp, accum_out=z_t[:])

            w1 = bp.tile([128, E * LP], BF16, tag="w1", name=f"w1{qq}")
            p_b = p_t[:].rearrange("p (o l) -> p o l", o=1).broadcast_to([128, E, LP])
            nc.vector.tensor_tensor(
                w1[:].rearrange("p (e l) -> p e l", e=E),
                vt_t[:].rearrange("p (e l) -> p e l", e=E),
                p_b, Alu.mult,
            )
            w2 = bp.tile([128, E * LP // 2], BF16, tag="w2", name=f"w2{qq}")
            w1v = w1[:].rearrange("p (e l) -> p e l", e=E)
            nc.vector.tensor_tensor(
                w2[:].rearrange("p (e l) -> p e l", e=E),
                w1v[:, :, 0:LP // 2], w1v[:, :, LP // 2:LP], Alu.add,
            )
            w3 = bp.tile([128, E * LP // 4], BF16, tag="w3", name=f"w3{qq}")
            w2v = w2[:].rearrange("p (e l) -> p e l", e=E)
            nc.vector.tensor_tensor(
                w3[:].rearrange("p (e l) -> p e l", e=E),
                w2v[:, :, 0:LP // 4], w2v[:, :, LP // 4:LP // 2], Alu.add,
            )
            acc = bp.tile([128, E], F32, tag="acc", name=f"acc{qq}")
            nc.vector.tensor_reduce(
                acc[:], w3[:].rearrange("p (e l) -> p e l", e=E), axis=X, op=Alu.add)
            rz = bp.tile([128, 1], F32, tag="rz", name=f"rz{qq}")
            nc.vector.reciprocal(rz[:], z_t[:])
            o_t = bp.tile([128, E], F32, tag="o", name=f"o{qq}")
            nc.vector.tensor_scalar_mul(o_t[:], acc[:], rz[:])
            nc.gpsimd.dma_start(out_d[qq * 128:(qq + 1) * 128, :], o_t[:])

        load_slab(0)
        st_t = None
        lg_t = None
        GP = 8  # pairs per h2 group ([80, 1024] f32 = 2 psum banks)
        slab_of_group = []
        for si, ng in enumerate(SLAB_GROUPS):
            slab_of_group += [si] * ng
        for g in range(32):
            s = slab_of_group[g]
            if g == SLAB_G0[s] and s + 1 < len(SLAB_GROUPS):
                load_slab(s + 1)
            if 2 <= g < 6:
                load_quarter(0, g - 2)
            elif g >= 8 and g % 8 < 4:
                load_quarter(g // 8, g % 8)

            kxs, vws = kx_t[s], vw_t[s]
            h2_t = h2p.tile([80, GP * LP], F32, tag="h2", name=f"h2_{g}")
            for pp in range(GP):
                rr = (g - SLAB_G0[s]) * GP + pp  # pair within slab
                nc.tensor.matmul(
                    h2_t[0:80, pp * LP:(pp + 1) * LP],
                    vws[:, rr * 80:rr * 80 + 80],
                    kxs[:, rr * LP:(rr + 1) * LP],
                    start=True, stop=True,
                )
            t_t = tp.tile([80, GP * LP], BF16, tag="t", name=f"t_{g}")
            nc.scalar.activation(t_t[:], h2_t[:], Tanh, scale=0.5)

            if g % 2 == 0:
                lg_t = lgp.tile([98, 512], F32, tag="lg", name=f"lg_{g // 2}")
            for j in range(2):
                ss = 2 * (g % 2) + j
                nc.tensor.matmul(
                    lg_t[32 * ss:32 * ss + 2, 0:512],
                    wf_t[:], t_t[:, 512 * j:512 * (j + 1)],
                    start=True, stop=True, tile_position=(0, 32 * ss),
                )
            if g % 2 == 1:
                qq, gq = g // 8, (g // 2) % 4
                if gq == 0:
                    st_t = stp.tile([98, 4 * 512], BF16, tag="st", name=f"st{qq}")
                nc.scalar.activation(
                    st_t[:, 512 * gq:512 * (gq + 1)], lg_t[:], Copy)

                if gq == 3:
                    lm_t = lmp.tile([128, LP], BF16, tag="lm", name=f"lm{qq}")
                    for ss in range(4):
                        # rows {32s, 32s+1} unfold to batch-major rows
                        # 32s..32s+32 in one DMA (row-major both sides)
                        nc.gpsimd.dma_start(
                            lm_t[32 * ss:32 * ss + 32, :],
                            st_t[32 * ss:32 * ss + 2, :])
                    phase_b(qq, lm_t)

    if not nc.is_finalized():
        nc.finalize()
    return nc


def host_prep(q, k, v, mask, W1, b1, W2, b2, Wf, bf):
    """Fold weights per batch, compact masked tokens, build device layouts."""
    q2 = q[:, 0, :].astype(np.float32)
    W1 = W1.astype(np.float32); W2 = W2.astype(np.float32)
    P = (W1[64:128] - W1[128:192]) @ W2                     # [64,40]
    Q = W1[192:256] @ W2                                    # [64,40]
    A2 = (W1[0:64] + W1[128:192]) @ W2
    c0 = b1.astype(np.float32) @ W2 + b2.astype(np.float32)
    r = q2 @ A2 + c0                                        # [Bf,40]
    V = P[None] + q2[:, :, None] * Q[None]                  # [Bf,64,40]
    G = np.einsum('beh,bei->bhi', V, V)
    y = np.linalg.solve(G, r[:, :, None])
    s = np.einsum('beh,bhx->be', V, y)                      # [Bf,64]

    m = mask[:, :, 0]
    order = np.argsort(m, axis=1, kind='stable')[:, :LP]
    nvalid = (~m).sum(1)
    assert nvalid.max() <= LP, f"batch with {nvalid.max()} unmasked tokens"
    validc = np.arange(LP)[None, :] < nvalid[:, None]       # [Bf,LP]
    kc = np.take_along_axis(k.astype(np.float32), order[:, :, None], 1)
    vc = np.take_along_axis(v.astype(np.float32), order[:, :, None], 1)
    kc = np.where(validc[..., None], kc + s[:, None, :], 0.0)
    vc = np.where(validc[..., None], vc, 0.0)
    maddf = np.where(validc, np.float32(0.0), np.float32(-30.0)).astype(nbf16)

    # core-local batch <-> (pair r, stream sig) map
    b = np.arange(B)
    qq = b // 128; t = b % 128
    s2s = t // 16; s_ = s2s // 2; sig = s2s % 2
    g_ = (t % 16) // 4; cb = t % 4
    r_ = 64 * qq + 16 * g_ + 4 * s_ + cb
    A_idx = np.empty(NPAIR, np.int64); B_idx = np.empty(NPAIR, np.int64)
    A_idx[r_[sig == 0]] = b[sig == 0]
    B_idx[r_[sig == 1]] = b[sig == 1]

    in_maps = []
    for c in range(N_CORES):
        sl = slice(c * B, (c + 1) * B)
        kcc, Vc = kc[sl], V[sl]
        kx = np.empty((128, NPAIR * LP), np.float32)
        kx[0:64] = kcc[A_idx].transpose(2, 0, 1).reshape(64, -1)
        kx[64:128] = kcc[B_idx].transpose(2, 0, 1).reshape(64, -1)
        vw3 = np.zeros((NPAIR, 128, 80), np.float32)
        vw3[:, 0:64, 0:40] = Vc[A_idx]
        vw3[:, 64:128, 40:80] = Vc[B_idx]
        vw = vw3.transpose(1, 0, 2).reshape(128, NPAIR * 80)
        vt = np.ascontiguousarray(vc[sl].transpose(0, 2, 1)).reshape(B, E * LP)
        wf2 = np.zeros((80, 2), np.float32)
        wf2[0:40, 0] = 0.5 * Wf[:, 0]
        wf2[40:80, 1] = 0.5 * Wf[:, 0]
        in_maps.append({
            "kx": np.ascontiguousarray(kx).astype(nbf16),
            "vw": np.ascontiguousarray(vw).astype(nbf16),
            "wf2": wf2.astype(nbf16),
            "vt": vt.astype(nbf16),
            "madd": np.ascontiguousarray(maddf[sl]),
        })
    return in_maps


_CACHE = {}


def run_on_device(in_maps, trace=False):
    if "nc" not in _CACHE:
        _CACHE["nc"] = build_nc()
    nc = _CACHE["nc"]
    res = run_bass_kernel_spmd(nc, in_maps, core_ids=list(range(N_CORES)),
                               trace=trace)
    return res


def kernel(q, k, v, mask, W1, b1, W2, b2, Wf, bf):
    in_maps = host_prep(q, k, v, mask, W1, b1, W2, b2, Wf, bf)
    res = run_on_device(in_maps)
    out = np.concatenate([res.results[c]["out"] for c in range(N_CORES)], axis=0)
    return out.astype(np.float32)
